# revision 47
# baseline (speedup 1.0000x reference)
"""Multi-head cosine self-attention on 8 Trainium2 NeuronCores (Bass/Tile).

Problem: y = MHA(x) with L2-normalized q/k (cosine attention) and per-head
scaling sim / n**sigmoid(m);  x: [4, 2048, 1024], 16 heads of dim 64.

KEY STRUCTURE: there is no softmax, so attention is LINEAR and associates:
    out_h = (q^ k^T / s_h) v = q^ @ (k^T v~) / s_h,   v~[j,:] = v[j,:]/||k_j||
Per head, M_h = k_h^T v~_h is only [64, 64] — the n x n similarity matrix is
never materialized, collapsing attention FLOPs 16x and eliminating the PSUM
eviction traffic that dominated the quadratic formulation.

Sharding: core c handles batch c//2 and head-group c%2 (8 heads = 512 of the
1024 q/k/v features).  Each core computes its partial output
(attn_out_part @ Wo[rows]); the host sums the two partials per batch and adds
bo.  No collectives.

Per-core pipeline (bf16 operands, fp32 PSUM):
  - k = x Wk + bk in NATURAL layout (rows j on partitions) like v; per-row
    norms ||k_j||_h via GpSimd square + DVE segmented reduce; 1/||k_j|| via
    DVE reciprocal + ACT Sqrt
  - v~ = v * (1/||k||) per head segment via one broadcast tensor_tensor per
    row-tile (stride-0 innermost AP)
  - M_h = k_h^T v~_h: 16 accumulating [K=128,M=64,N=64] matmuls per head,
    2 heads col-packed per PSUM tile; evicted with ACT scale 1/s_h (and the
    per-head n^sigmoid(m) folded in)
  - qT = (x Wq + bq)^T (features on partitions, 2 heads per 128-partition
    tile); normalized in place via the [2,n] column-norm matmul trick,
    reciprocal+sqrt, a K=2 indicator broadcast matmul, and DVE multiply
  - attn_out^T = M_h^T @ q^T: M stationary, q^T streams N=512; two heads
    row+col packed per PSUM tile -> same transposed layout the output
    projection expects
  - out = attn_out^T.T @ Wo_rows, evicted and DMA'd per 128-row tile
Biases are added with K=1 outer-product matmuls inside each projection's
accumulation group, so every PSUM eviction is a plain copy assigned to DVE or
ACT by a greedy running-load balance.
"""

import os
import sys

for _p in ("/opt/trn_rl_repo",):
    if os.path.isdir(_p) and _p not in sys.path:
        sys.path.insert(0, _p)

from contextlib import ExitStack

import ml_dtypes
import numpy as np

import concourse.bacc as bacc
import concourse.mybir as mybir
import concourse.tile as tile
from concourse import bass_utils

P = 128
F = 1024  # model dim
H = 16  # total heads
HD = 64  # head dim
G = 2  # head groups (tensor-parallel factor)
FG = F // G  # 512 features per core
PAIRS = FG // P  # 4 head-pairs per core
NSEG = FG // HD  # 8 head segments per core
KT = F // P  # 8 contraction tiles for the projections
NCORES = 8
F32 = mybir.dt.float32
BF = mybir.dt.bfloat16
AF = mybir.ActivationFunctionType
MUL = mybir.AluOpType.mult


def _mm(nc, out, lhsT, rhs, **kw):
    nc.tensor.matmul(out, lhsT, rhs, **kw)


def build_core_program(nc, n=2048, has_bias=True):
    NC = n // 512  # 512-row chunks
    NT = n // P  # 128-row tiles

    xt = nc.dram_tensor("xt", [P, NC, KT, 512], BF, kind="ExternalInput").ap()
    wq = nc.dram_tensor("wq", [P, PAIRS, KT, P], BF, kind="ExternalInput").ap()
    wk = nc.dram_tensor("wk", [P, KT, FG], BF, kind="ExternalInput").ap()
    wv = nc.dram_tensor("wv", [P, KT, FG], BF, kind="ExternalInput").ap()
    wo = nc.dram_tensor("wo", [P, PAIRS, F], BF, kind="ExternalInput").ap()
    if has_bias:
        bqr = nc.dram_tensor("bqr", [1, FG], BF, kind="ExternalInput").ap()
        bkr = nc.dram_tensor("bkr", [1, FG], BF, kind="ExternalInput").ap()
        bvr = nc.dram_tensor("bvr", [1, FG], BF, kind="ExternalInput").ap()
    # sinv[p, pr] = n**-sigmoid(m_h) for local head h = 2*pr + (p >= 64)
    sinv = nc.dram_tensor("sinv", [P, PAIRS], F32, kind="ExternalInput").ap()
    cind = nc.dram_tensor("cind", [2, P], BF, kind="ExternalInput").ap()
    cblk = nc.dram_tensor("cblk", [P, 2], BF, kind="ExternalInput").ap()
    out = nc.dram_tensor("out", [n, F], BF, kind="ExternalOutput").ap()

    with tile.TileContext(nc) as tc, ExitStack() as ctx:
        const = ctx.enter_context(tc.tile_pool(name="const", bufs=1))
        persist = ctx.enter_context(tc.tile_pool(name="persist", bufs=1))
        work = ctx.enter_context(tc.tile_pool(name="work", bufs=1))
        ps = ctx.enter_context(tc.tile_pool(name="ps", bufs=1, space="PSUM"))

        # --- greedy DVE/ACT load balance for PSUM evictions --------------
        load = {"dve": 0.0, "act": 0.0}

        def copy_ps(dst, src, fd):
            dve_c = (120 + fd) / 0.96
            act_c = (172 + fd) / 1.2
            if load["dve"] + dve_c <= load["act"] + act_c:
                load["dve"] += dve_c
                nc.vector.tensor_copy(dst, src)
            else:
                load["act"] += act_c
                nc.scalar.copy(dst, src)

        # --- persistent tensors ------------------------------------------
        qT = persist.tile([P, PAIRS, n], BF)  # (x Wq + bq)^T, 2 heads/tile
        k_nat = persist.tile([P, NT, FG], BF)  # x Wk + bk, natural layout
        v = persist.tile([P, NT, FG], BF)  # (x Wv + bv) / ||k|| per segment
        aoT = persist.tile([P, PAIRS, n], BF)  # attn-out^T
        rk = persist.tile([P, NSEG, NT], BF)  # 1/||k_j|| per head segment
        # M^ = (k^T v~)/s as a block-diagonal [128,128] per pair (off-diag
        # zeroed) so attn-out is a single unpacked K=128 matmul
        mh = persist.tile([P, PAIRS, P], BF)
        xall = persist.tile([P, NC, KT, 512], BF)
        wk_sb = persist.tile([P, KT, FG], BF)
        wv_sb = persist.tile([P, KT, FG], BF)
        wo_sb = persist.tile([P, PAIRS, F], BF)

        def emit_wq_dma(pr):
            wf = work.tile([P, KT, P], BF, tag="wqf", bufs=2)
            nc.sync.dma_start(wf[:], wq[:, pr])
            return wf

        # --- constants with no DMA dependency ----------------------------
        ones512 = const.tile([1, 512], BF)
        nc.any.memset(ones512[:], 1.0)
        zcol = const.tile([P, 1], F32)
        nc.any.memset(zcol[:], 0.0)
        nc.any.memset(mh[:], 0.0)  # off-diagonal blocks stay zero

        # --- loads: two queues, strict need-order.  The first k-nat tile
        # needs wk + x chunk 0 + bk_row (1 KB, slotted between the x0
        # halves); every later tensor queues behind in need-order so it
        # does not steal HBM bandwidth from the critical set.  Tiny const
        # DMAs pay ~1us SWDGE first-byte each, so they ride at the back —
        # all are consumed far later than the total DMA span.
        ones_blk = const.tile([P, 2], BF)
        ind = const.tile([2, P], BF)
        sinv_sb = const.tile([P, PAIRS], F32)
        if has_bias:
            bq_row = const.tile([1, FG], BF)
            bk_row = const.tile([1, FG], BF)
            bv_row = const.tile([1, FG], BF)

        quart = KT // 4
        half = KT // 2
        nc.sync.dma_start(wk_sb[:, :quart], wk[:, :quart])
        nc.scalar.dma_start(xall[:, 0, :quart], xt[:, 0, :quart])
        nc.sync.dma_start(wk_sb[:, quart:half], wk[:, quart:half])
        nc.scalar.dma_start(xall[:, 0, quart:half], xt[:, 0, quart:half])
        if has_bias:
            nc.scalar.dma_start(bk_row[:], bkr)
        nc.sync.dma_start(wk_sb[:, half:], wk[:, half:])
        nc.scalar.dma_start(xall[:, 0, half:], xt[:, 0, half:])
        nc.sync.dma_start(xall[:, 1], xt[:, 1])
        nc.scalar.dma_start(xall[:, 2], xt[:, 2])
        nc.sync.dma_start(xall[:, 3], xt[:, 3])
        nc.scalar.dma_start(wv_sb[:], wv)
        if has_bias:
            nc.scalar.dma_start(bv_row[:], bvr)
        wf0 = emit_wq_dma(0)
        nc.scalar.dma_start(wo_sb[:], wo)
        if has_bias:
            nc.sync.dma_start(bq_row[:], bqr)
        nc.sync.dma_start(ones_blk[:], cblk)
        nc.sync.dma_start(ind[:], cind)
        nc.sync.dma_start(sinv_sb[:], sinv)

        def emit_nat_tile(jt, w_sb, brow, dst):
            # one 128-row tile of a natural-layout projection x W + b
            ic, t = divmod(jt, NT // NC)
            jsl = slice(t * P, (t + 1) * P)
            pt = ps.tile([P, FG], F32, tag="mm", bufs=5)
            for k in range(KT):
                _mm(nc, pt, xall[:, ic, k, jsl], w_sb[:, k, :],
                    start=(k == 0), stop=(not has_bias and k == KT - 1))
            if has_bias:
                _mm(nc, pt, ones512[:, :P], brow, start=False, stop=True)
            copy_ps(dst[:, jt, :], pt, 512)

        def emit_knorm(jt):
            # 1/||k_j|| per head segment for one 128-row tile
            ksq = work.tile([P, FG], BF, tag="ksq", bufs=2)
            nc.gpsimd.tensor_tensor(ksq[:], k_nat[:, jt, :], k_nat[:, jt, :], MUL)
            nksq = work.tile([P, NSEG], F32, tag="nksq", bufs=2)
            nc.vector.tensor_reduce(nksq[:],
                                    ksq[:].rearrange("p (s d) -> p s d", s=NSEG),
                                    mybir.AxisListType.X, mybir.AluOpType.add)
            load["dve"] += 658
            rk1 = work.tile([P, NSEG], F32, tag="rk1", bufs=2)
            nc.vector.reciprocal(rk1[:], nksq[:])
            load["dve"] += 140
            nc.scalar.activation(rk[:, :, jt], rk1, AF.Sqrt, bias=zcol[:])
            load["act"] += 200

        def emit_vscale(jt):
            # v~[j,:] = v[j,:] * rk[j, seg] with a stride-0 innermost view
            rkb = rk[:, :, jt:jt + 1].to_broadcast([P, NSEG, HD])
            vv = v[:, jt, :].rearrange("p (s d) -> p s d", s=NSEG)
            if jt % 2 == 0:
                nc.vector.tensor_tensor(vv, vv, rkb, MUL)
                load["dve"] += 690
            else:
                nc.gpsimd.tensor_tensor(vv, vv, rkb, MUL)

        def emit_q_chunk(pr, wf, ic):
            # transposed q projection of pair pr for one 512-row chunk
            isl = slice(ic * 512, (ic + 1) * 512)
            pt = ps.tile([P, 512], F32, tag="mm", bufs=5)
            for k in range(KT):
                _mm(nc, pt, wf[:, k, :], xall[:, ic, k, :],
                    start=(k == 0), stop=(not has_bias and k == KT - 1))
            if has_bias:
                _mm(nc, pt, bq_row[:, pr * P:(pr + 1) * P], ones512,
                    start=False, stop=True)
            copy_ps(qT[:, pr, isl], pt, 512)

        def emit_qnorm(pr):
            # in-place q^ = q/||q|| via the column-norm matmul trick;
            # the square is chunked and alternates Pool/DVE to keep the
            # chain latency off the PE critical path
            sq = work.tile([P, n], BF, tag="sq", bufs=2)
            rowr = work.tile([2, n], BF, tag="rowr", bufs=2)
            for ch in range(NC):
                csl = slice(ch * 512, (ch + 1) * 512)
                if ch % 2 == 0:
                    nc.gpsimd.tensor_tensor(sq[:, csl], qT[:, pr, csl],
                                            qT[:, pr, csl], MUL)
                else:
                    nc.vector.tensor_tensor(sq[:, csl], qT[:, pr, csl],
                                            qT[:, pr, csl], MUL)
                    load["dve"] += 424
                nps = ps.tile([2, 512], F32, tag="mm", bufs=5)
                _mm(nc, nps, ones_blk, sq[:, csl], start=True, stop=True)
                row = work.tile([2, 512], F32, tag="row", bufs=2)
                nc.vector.reciprocal(row[:], nps)
                load["dve"] += 658
                nc.scalar.activation(rowr[:, csl], row, AF.Sqrt, bias=zcol[:2])
                load["act"] += 570
            for ch in range(NC):
                csl = slice(ch * 512, (ch + 1) * 512)
                bps = ps.tile([P, 512], F32, tag="mm", bufs=5)
                _mm(nc, bps, ind, rowr[:, csl], start=True, stop=True)
                nc.vector.tensor_tensor(qT[:, pr, csl], qT[:, pr, csl],
                                        bps, MUL)
                load["dve"] += 690

        def emit_m(pr):
            # whole-pair M = k_pair^T v~_pair in one K=128 matmul per row
            # tile; the useful blocks are the diagonal ones (k_a^T v_a,
            # k_b^T v_b) — the off-diagonal junk is discarded at eviction
            mps = ps.tile([P, P], F32, tag="mps", bufs=1)
            psl = slice(pr * P, (pr + 1) * P)
            for jt in range(NT):
                _mm(nc, mps, k_nat[:, jt, psl], v[:, jt, psl],
                    start=(jt == 0), stop=(jt == NT - 1))
            # evictions fold the per-head 1/n**sigmoid(m) scale
            nc.scalar.activation(mh[0:HD, pr, 0:HD], mps[0:HD, 0:HD],
                                 AF.Identity, bias=zcol[:HD],
                                 scale=sinv_sb[0:HD, pr:pr + 1])
            nc.scalar.activation(mh[HD:P, pr, HD:P], mps[HD:P, HD:P],
                                 AF.Identity, bias=zcol[HD:P],
                                 scale=sinv_sb[HD:P, pr:pr + 1])
            load["act"] += 360

        def emit_ao_chunk(pr, ic):
            # attn_out^T = M^.T @ q^T, one unpacked K=128 matmul per chunk
            isl = slice(ic * 512, (ic + 1) * 512)
            ap2 = ps.tile([P, 512], F32, tag="av", bufs=2)
            _mm(nc, ap2, mh[:, pr, :], qT[:, pr, isl],
                start=True, stop=True)
            copy_ps(aoT[:, pr, isl], ap2, 512)

        def emit_out_chunk(ic):
            # final projection for one 512-row chunk; DMA queues alternate
            for t in range(NT // NC):
                nt = ic * (NT // NC) + t
                ntsl = slice(nt * P, (nt + 1) * P)
                ost = work.tile([P, F], BF, tag="ost", bufs=3)
                for fc in range(F // 512):
                    fsl = slice(fc * 512, (fc + 1) * 512)
                    pt2 = ps.tile([P, 512], F32, tag="mm", bufs=5)
                    for kt in range(PAIRS):
                        _mm(nc, pt2, aoT[:, kt, ntsl], wo_sb[:, kt, fsl],
                            start=(kt == 0), stop=(kt == PAIRS - 1))
                    copy_ps(ost[:, fsl], pt2, 512)
                if nt == NT - 1:
                    # final tile: halve the trailing DMA via two queues
                    nc.sync.dma_start(out[ntsl, :512], ost[:, :512])
                    nc.scalar.dma_start(out[ntsl, 512:], ost[:, 512:])
                else:
                    eng = (nc.sync, nc.scalar, nc.gpsimd)[nt % 3]
                    eng.dma_start(out[ntsl, :], ost[:])

        # ================= emission ======================================
        for jt in range(NT):
            emit_nat_tile(jt, wk_sb, bk_row if has_bias else None, k_nat)
            emit_knorm(jt)
        for jt in range(NT):
            emit_nat_tile(jt, wv_sb, bv_row if has_bias else None, v)
            emit_vscale(jt)

        # pair loop software-pipelined: pair pr's M/ao matmuls are emitted
        # AFTER pair pr+1's norm chain so they (and pr+2's q projection)
        # fill the PE while that chain runs on Pool/DVE/ACT
        for ic in range(NC):
            emit_q_chunk(0, wf0, ic)
        wf = emit_wq_dma(1)
        for ic in range(NC):
            emit_q_chunk(1, wf, ic)
        emit_qnorm(0)
        for pr in range(PAIRS):
            if pr + 2 < PAIRS:
                wf = emit_wq_dma(pr + 2)
                for ic in range(NC):
                    emit_q_chunk(pr + 2, wf, ic)
            if pr + 1 < PAIRS:
                emit_qnorm(pr + 1)
            if pr < PAIRS - 1:
                emit_m(pr)
                if pr == PAIRS - 2:
                    emit_m(pr + 1)  # extra PE filler for the last norm chain
            if pr < PAIRS - 1:
                for ic in range(NC):
                    emit_ao_chunk(pr, ic)
            else:
                # last pair: each attn-out chunk immediately unblocks the
                # output projection for that chunk, filling the tail
                for ic in range(NC):
                    emit_ao_chunk(pr, ic)
                    emit_out_chunk(ic)
    return nc


_CACHE = {}


def get_nc(n=2048, has_bias=True):
    key = (n, has_bias)
    if key not in _CACHE:
        nc = bacc.Bacc("TRN2", target_bir_lowering=False, debug=False,
                       num_devices=NCORES)
        build_core_program(nc, n, has_bias=has_bias)
        nc.compile()
        _CACHE[key] = nc
    return _CACHE[key]


def _warr(W, sl):
    return np.ascontiguousarray(
        np.asarray(W, np.float32)[:, sl].reshape(KT, P, FG)
        .transpose(1, 0, 2)).astype(ml_dtypes.bfloat16)


def _warr_ft(W, sl):
    return np.ascontiguousarray(
        np.asarray(W, np.float32)[:, sl].reshape(KT, P, PAIRS, P)
        .transpose(1, 2, 0, 3)).astype(ml_dtypes.bfloat16)


_IND = np.zeros((2, P), ml_dtypes.bfloat16)
_IND[0, :HD] = 1.0
_IND[1, HD:] = 1.0
_BLK = np.zeros((P, 2), ml_dtypes.bfloat16)
_BLK[:HD, 0] = 1.0
_BLK[HD:, 1] = 1.0
_ONES = np.ones((1, 512), ml_dtypes.bfloat16)


def make_in_maps(x, Wq, bq, Wk, bk, Wv, bv, Wo, bo, m, has_bias=True):
    n = x.shape[1]
    sig = 1.0 / (1.0 + np.exp(-np.asarray(m, np.float64)))
    scale = np.float64(n) ** sig  # [16] per-head n^sigmoid(m)
    in_maps = []
    for c in range(NCORES):
        bi, g = divmod(c, 2)
        sl = slice(g * FG, (g + 1) * FG)
        hsc = scale[g * (H // G):(g + 1) * (H // G)]  # 8 local heads
        sv = np.empty((P, PAIRS), np.float32)
        for pr in range(PAIRS):
            sv[:HD, pr] = 1.0 / hsc[2 * pr]
            sv[HD:, pr] = 1.0 / hsc[2 * pr + 1]
        xa = np.asarray(x[bi], np.float32)
        NCc = n // 512
        im = {
            "xt": np.ascontiguousarray(
                xa.reshape(NCc, 512, KT, P).transpose(3, 0, 2, 1))
                .astype(ml_dtypes.bfloat16),
            "wq": _warr_ft(Wq, sl), "wk": _warr(Wk, sl), "wv": _warr(Wv, sl),
            "wo": np.ascontiguousarray(
                np.asarray(Wo, np.float32)[sl].reshape(PAIRS, P, F)
                .transpose(1, 0, 2).astype(ml_dtypes.bfloat16)),
            "sinv": sv,
            "cind": _IND,
            "cblk": _BLK,
        }
        if has_bias:
            im["bqr"] = np.asarray(bq, np.float32)[sl].reshape(1, FG).astype(ml_dtypes.bfloat16)
            im["bkr"] = np.asarray(bk, np.float32)[sl].reshape(1, FG).astype(ml_dtypes.bfloat16)
            im["bvr"] = np.asarray(bv, np.float32)[sl].reshape(1, FG).astype(ml_dtypes.bfloat16)
        in_maps.append(im)
    return in_maps


def kernel(x, Wq, bq, Wk, bk, Wv, bv, Wo, bo, m, _trace=False):
    x = np.asarray(x, np.float32)
    b, n, f = x.shape
    has_bias = bool(np.any(np.asarray(bq)) or np.any(np.asarray(bk))
                    or np.any(np.asarray(bv)))
    nc = get_nc(n, has_bias)
    in_maps = make_in_maps(x, Wq, bq, Wk, bk, Wv, bv, Wo, bo, m,
                           has_bias=has_bias)
    res = bass_utils.run_bass_kernel_spmd(nc, in_maps,
                                          core_ids=list(range(NCORES)),
                                          trace=_trace)
    outs = [r["out"] for r in res.results]
    y = np.empty((b, n, f), np.float32)
    for bi in range(b):
        y[bi] = outs[2 * bi].astype(np.float32) + outs[2 * bi + 1].astype(np.float32)
    y += np.asarray(bo, np.float32).reshape(1, 1, f)
    if _trace:
        kernel._last_results = res
    return y


if __name__ == "__main__":
    # build-only smoke test (no device)
    nc = bacc.Bacc("TRN2", target_bir_lowering=False, debug=False,
                   num_devices=NCORES)
    build_core_program(nc, n=int(sys.argv[1]) if len(sys.argv) > 1 else 2048)
    print("build OK")


# revision 66
# speedup vs baseline: 1.1707x; 1.1707x over previous
"""Multi-head cosine self-attention on 8 Trainium2 NeuronCores (Bass/Tile).

Problem: y = MHA(x) with L2-normalized q/k (cosine attention) and per-head
scaling sim / n**sigmoid(m);  x: [4, 2048, 1024], 16 heads of dim 64.

KEY STRUCTURE: there is no softmax, so attention is LINEAR and associates:
    out_h = (q^ k^T / s_h) v = q^ @ (k^T v~) / s_h,   v~[j,:] = v[j,:]/||k_j||
Per head, M_h = k_h^T v~_h is only [64, 64] — the n x n similarity matrix is
never materialized, collapsing attention FLOPs 16x and eliminating the PSUM
eviction traffic that dominated the quadratic formulation.

Sharding: core c handles batch c//2 and head-group c%2 (8 heads = 512 of the
1024 q/k/v features).  Each core computes its partial output
(attn_out_part @ Wo[rows]); the host sums the two partials per batch and adds
bo.  No collectives.

Per-core pipeline (bf16 operands, fp32 PSUM):
  - k = x Wk + bk in NATURAL layout (rows j on partitions) like v; per-row
    norms ||k_j||_h via GpSimd square + DVE segmented reduce; 1/||k_j|| via
    DVE reciprocal + ACT Sqrt
  - v~ = v * (1/||k||) per head segment via one broadcast tensor_tensor per
    row-tile (stride-0 innermost AP)
  - M_h = k_h^T v~_h: 16 accumulating [K=128,M=64,N=64] matmuls per head,
    2 heads col-packed per PSUM tile; evicted with ACT scale 1/s_h (and the
    per-head n^sigmoid(m) folded in)
  - qT = (x Wq + bq)^T (features on partitions, 2 heads per 128-partition
    tile); normalized in place via the [2,n] column-norm matmul trick,
    reciprocal+sqrt, a K=2 indicator broadcast matmul, and DVE multiply
  - attn_out^T = M_h^T @ q^T: M stationary, q^T streams N=512; two heads
    row+col packed per PSUM tile -> same transposed layout the output
    projection expects
  - out = attn_out^T.T @ Wo_rows, evicted and DMA'd per 128-row tile
Biases are added with K=1 outer-product matmuls inside each projection's
accumulation group, so every PSUM eviction is a plain copy assigned to DVE or
ACT by a greedy running-load balance.
"""

import os
import sys

for _p in ("/opt/trn_rl_repo",):
    if os.path.isdir(_p) and _p not in sys.path:
        sys.path.insert(0, _p)

from contextlib import ExitStack

import ml_dtypes
import numpy as np

import concourse.bacc as bacc
import concourse.mybir as mybir
import concourse.tile as tile
from concourse import bass_utils

P = 128
F = 1024  # model dim
H = 16  # total heads
HD = 64  # head dim
G = 2  # head groups (tensor-parallel factor)
FG = F // G  # 512 features per core
PAIRS = FG // P  # 4 head-pairs per core
NSEG = FG // HD  # 8 head segments per core
KT = F // P  # 8 contraction tiles for the projections
NCORES = 8
F32 = mybir.dt.float32
BF = mybir.dt.bfloat16
AF = mybir.ActivationFunctionType
MUL = mybir.AluOpType.mult


def _mm(nc, out, lhsT, rhs, **kw):
    nc.tensor.matmul(out, lhsT, rhs, **kw)


def build_core_program(nc, n=2048, has_bias=True):
    NC = n // 512  # 512-row chunks
    NT = n // P  # 128-row tiles

    xt = nc.dram_tensor("xt", [P, NC, KT, 512], BF, kind="ExternalInput").ap()
    wq = nc.dram_tensor("wq", [P, PAIRS, KT, P], BF, kind="ExternalInput").ap()
    wk = nc.dram_tensor("wk", [P, KT, FG], BF, kind="ExternalInput").ap()
    wv = nc.dram_tensor("wv", [P, KT, FG], BF, kind="ExternalInput").ap()
    wo = nc.dram_tensor("wo", [P, PAIRS, F], BF, kind="ExternalInput").ap()
    if has_bias:
        bqr = nc.dram_tensor("bqr", [1, FG], BF, kind="ExternalInput").ap()
        bkr = nc.dram_tensor("bkr", [1, FG], BF, kind="ExternalInput").ap()
        bvr = nc.dram_tensor("bvr", [1, FG], BF, kind="ExternalInput").ap()
    # sinv[p, pr] = n**-sigmoid(m_h) for local head h = 2*pr + (p >= 64)
    sinv = nc.dram_tensor("sinv", [P, PAIRS], F32, kind="ExternalInput").ap()
    cind = nc.dram_tensor("cind", [2, P], BF, kind="ExternalInput").ap()
    cblk = nc.dram_tensor("cblk", [P, 2], BF, kind="ExternalInput").ap()
    out = nc.dram_tensor("out", [n, F], BF, kind="ExternalOutput").ap()

    with tile.TileContext(nc) as tc, ExitStack() as ctx:
        const = ctx.enter_context(tc.tile_pool(name="const", bufs=1))
        persist = ctx.enter_context(tc.tile_pool(name="persist", bufs=1))
        work = ctx.enter_context(tc.tile_pool(name="work", bufs=1))
        ps = ctx.enter_context(tc.tile_pool(name="ps", bufs=1, space="PSUM"))

        # --- greedy DVE/ACT load balance for PSUM evictions --------------
        load = {"dve": 0.0, "act": 0.0}

        def copy_ps(dst, src, fd):
            dve_c = (120 + fd) / 0.96
            act_c = (172 + fd) / 1.2
            if load["dve"] + dve_c <= load["act"] + act_c:
                load["dve"] += dve_c
                nc.vector.tensor_copy(dst, src)
            else:
                load["act"] += act_c
                nc.scalar.copy(dst, src)

        # --- persistent tensors ------------------------------------------
        qT = persist.tile([P, PAIRS, n], BF)  # (x Wq + bq)^T, 2 heads/tile
        k_nat = persist.tile([P, NT, FG], BF)  # x Wk + bk, natural layout
        v = persist.tile([P, NT, FG], BF)  # (x Wv + bv) / ||k|| per segment
        wP = persist.tile([P, PAIRS, F], BF)  # W' = M^ @ Wo per pair
        rk = persist.tile([P, NSEG, NT], BF)  # 1/||k_j|| per head segment
        # M^ = (k^T v~)/s as a block-diagonal [128,128] per pair (off-diag
        # zeroed) so attn-out is a single unpacked K=128 matmul
        mh = persist.tile([P, PAIRS, P], BF)
        xall = persist.tile([P, NC, KT, 512], BF)
        wk_sb = persist.tile([P, KT, FG], BF)
        wv_sb = persist.tile([P, KT, FG], BF)
        wo_sb = persist.tile([P, PAIRS, F], BF)

        def emit_wq_dma(pr):
            wf = work.tile([P, KT, P], BF, tag="wqf", bufs=2)
            nc.sync.dma_start(wf[:], wq[:, pr])
            return wf

        # --- constants with no DMA dependency ----------------------------
        ones512 = const.tile([1, 512], BF)
        nc.any.memset(ones512[:], 1.0)
        zcol = const.tile([P, 1], F32)
        nc.any.memset(zcol[:], 0.0)
        nc.any.memset(mh[:], 0.0)  # off-diagonal blocks stay zero

        # --- loads: two queues, strict need-order.  The first k-nat tile
        # needs wk + x chunk 0 + bk_row (1 KB, slotted between the x0
        # halves); every later tensor queues behind in need-order so it
        # does not steal HBM bandwidth from the critical set.  Tiny const
        # DMAs pay ~1us SWDGE first-byte each, so they ride at the back —
        # all are consumed far later than the total DMA span.
        ones_blk = const.tile([P, 2], BF)
        ind = const.tile([2, P], BF)
        sinv_sb = const.tile([P, PAIRS], F32)
        if has_bias:
            bq_row = const.tile([1, FG], BF)
            bk_row = const.tile([1, FG], BF)
            bv_row = const.tile([1, FG], BF)

        quart = KT // 4
        half = KT // 2
        nc.sync.dma_start(wk_sb[:, :quart], wk[:, :quart])
        nc.scalar.dma_start(xall[:, 0, :quart], xt[:, 0, :quart])
        nc.sync.dma_start(wk_sb[:, quart:half], wk[:, quart:half])
        nc.scalar.dma_start(xall[:, 0, quart:half], xt[:, 0, quart:half])
        if has_bias:
            nc.scalar.dma_start(bk_row[:], bkr)
        nc.sync.dma_start(wk_sb[:, half:], wk[:, half:])
        nc.scalar.dma_start(xall[:, 0, half:], xt[:, 0, half:])
        nc.sync.dma_start(xall[:, 1], xt[:, 1])
        nc.scalar.dma_start(xall[:, 2], xt[:, 2])
        nc.sync.dma_start(xall[:, 3], xt[:, 3])
        nc.scalar.dma_start(wv_sb[:], wv)
        if has_bias:
            nc.scalar.dma_start(bv_row[:], bvr)
        wf0 = emit_wq_dma(0)
        nc.scalar.dma_start(wo_sb[:], wo)
        if has_bias:
            nc.sync.dma_start(bq_row[:], bqr)
        nc.sync.dma_start(ones_blk[:], cblk)
        nc.sync.dma_start(ind[:], cind)
        nc.sync.dma_start(sinv_sb[:], sinv)

        def emit_nat_tile(jt, w_sb, brow, dst):
            # one 128-row tile of a natural-layout projection x W + b
            ic, t = divmod(jt, NT // NC)
            jsl = slice(t * P, (t + 1) * P)
            pt = ps.tile([P, FG], F32, tag="mm", bufs=5)
            for k in range(KT):
                _mm(nc, pt, xall[:, ic, k, jsl], w_sb[:, k, :],
                    start=(k == 0), stop=(not has_bias and k == KT - 1))
            if has_bias:
                _mm(nc, pt, ones512[:, :P], brow, start=False, stop=True)
            copy_ps(dst[:, jt, :], pt, 512)

        def emit_knorm(jt):
            # 1/||k_j|| per head segment for one 128-row tile
            ksq = work.tile([P, FG], BF, tag="ksq", bufs=2)
            nc.gpsimd.tensor_tensor(ksq[:], k_nat[:, jt, :], k_nat[:, jt, :], MUL)
            nksq = work.tile([P, NSEG], F32, tag="nksq", bufs=2)
            nc.vector.tensor_reduce(nksq[:],
                                    ksq[:].rearrange("p (s d) -> p s d", s=NSEG),
                                    mybir.AxisListType.X, mybir.AluOpType.add)
            load["dve"] += 658
            rk1 = work.tile([P, NSEG], F32, tag="rk1", bufs=2)
            nc.vector.reciprocal(rk1[:], nksq[:])
            load["dve"] += 140
            nc.scalar.activation(rk[:, :, jt], rk1, AF.Sqrt, bias=zcol[:])
            load["act"] += 200

        def emit_vscale(jt):
            # v~[j,:] = v[j,:] * rk[j, seg] with a stride-0 innermost view
            rkb = rk[:, :, jt:jt + 1].to_broadcast([P, NSEG, HD])
            vv = v[:, jt, :].rearrange("p (s d) -> p s d", s=NSEG)
            if jt % 2 == 0:
                nc.vector.tensor_tensor(vv, vv, rkb, MUL)
                load["dve"] += 690
            else:
                nc.gpsimd.tensor_tensor(vv, vv, rkb, MUL)

        def emit_q_chunk(pr, wf, ic):
            # transposed q projection of pair pr for one 512-row chunk
            isl = slice(ic * 512, (ic + 1) * 512)
            pt = ps.tile([P, 512], F32, tag="mm", bufs=5)
            for k in range(KT):
                _mm(nc, pt, wf[:, k, :], xall[:, ic, k, :],
                    start=(k == 0), stop=(not has_bias and k == KT - 1))
            if has_bias:
                _mm(nc, pt, bq_row[:, pr * P:(pr + 1) * P], ones512,
                    start=False, stop=True)
            copy_ps(qT[:, pr, isl], pt, 512)

        def emit_qnorm(pr):
            # in-place q^ = q/||q|| via the column-norm matmul trick;
            # the square is chunked and alternates Pool/DVE to keep the
            # chain latency off the PE critical path
            sq = work.tile([P, n], BF, tag="sq", bufs=2)
            rowr = work.tile([2, n], BF, tag="rowr", bufs=2)
            for ch in range(NC):
                csl = slice(ch * 512, (ch + 1) * 512)
                if ch % 2 == 0:
                    nc.gpsimd.tensor_tensor(sq[:, csl], qT[:, pr, csl],
                                            qT[:, pr, csl], MUL)
                else:
                    nc.vector.tensor_tensor(sq[:, csl], qT[:, pr, csl],
                                            qT[:, pr, csl], MUL)
                    load["dve"] += 424
                nps = ps.tile([2, 512], F32, tag="av", bufs=2)
                _mm(nc, nps, ones_blk, sq[:, csl], start=True, stop=True)
                row = work.tile([2, 512], F32, tag="row", bufs=2)
                nc.vector.reciprocal(row[:], nps)
                load["dve"] += 658
                nc.scalar.activation(rowr[:, csl], row, AF.Sqrt, bias=zcol[:2])
                load["act"] += 570
            for ch in range(NC):
                csl = slice(ch * 512, (ch + 1) * 512)
                bps = ps.tile([P, 512], F32, tag="av", bufs=2)
                _mm(nc, bps, ind, rowr[:, csl], start=True, stop=True)
                nc.vector.tensor_tensor(qT[:, pr, csl], qT[:, pr, csl],
                                        bps, MUL)
                load["dve"] += 690

        def emit_m(pr):
            # whole-pair M^T = v~_pair^T k_pair in one K=128 matmul per row
            # tile; the useful blocks are the diagonal ones — the
            # off-diagonal junk is discarded at eviction
            mps = ps.tile([P, P], F32, tag="mps", bufs=1)
            psl = slice(pr * P, (pr + 1) * P)
            for jt in range(NT):
                _mm(nc, mps, v[:, jt, psl], k_nat[:, jt, psl],
                    start=(jt == 0), stop=(jt == NT - 1))
            # evictions fold the per-head 1/n**sigmoid(m) scale
            nc.scalar.activation(mh[0:HD, pr, 0:HD], mps[0:HD, 0:HD],
                                 AF.Identity, bias=zcol[:HD],
                                 scale=sinv_sb[0:HD, pr:pr + 1])
            nc.scalar.activation(mh[HD:P, pr, HD:P], mps[HD:P, HD:P],
                                 AF.Identity, bias=zcol[HD:P],
                                 scale=sinv_sb[HD:P, pr:pr + 1])
            load["act"] += 360

        def emit_wp(pr):
            # fold M^ into the output weights: W'_pr = M^_pr @ Wo_pr.
            # out = aoT^T Wo = (M^T qT)^T Wo = qT^T (M^ Wo) — the attn-out
            # stage and its evictions vanish; mh holds M^T block-diagonal
            # so lhsT = mh gives the d-contraction directly.
            for fc in range(F // 512):
                fsl = slice(fc * 512, (fc + 1) * 512)
                wps = ps.tile([P, 512], F32, tag="av", bufs=2)
                _mm(nc, wps, mh[:, pr, :], wo_sb[:, pr, fsl],
                    start=True, stop=True)
                copy_ps(wP[:, pr, fsl], wps, 512)

        def emit_out_chunk(ic):
            # final projection for one 512-row chunk; DMA queues alternate
            for t in range(NT // NC):
                nt = ic * (NT // NC) + t
                ntsl = slice(nt * P, (nt + 1) * P)
                ost = work.tile([P, F], BF, tag="ost", bufs=3)
                for fc in range(F // 512):
                    fsl = slice(fc * 512, (fc + 1) * 512)
                    pt2 = ps.tile([P, 512], F32, tag="mm", bufs=5)
                    for kt in range(PAIRS):
                        _mm(nc, pt2, qT[:, kt, ntsl], wP[:, kt, fsl],
                            start=(kt == 0), stop=(kt == PAIRS - 1))
                    copy_ps(ost[:, fsl], pt2, 512)
                if nt == NT - 1:
                    # final tile: halve the trailing DMA via two queues
                    nc.sync.dma_start(out[ntsl, :512], ost[:, :512])
                    nc.scalar.dma_start(out[ntsl, 512:], ost[:, 512:])
                else:
                    eng = (nc.sync, nc.scalar, nc.gpsimd)[nt % 3]
                    eng.dma_start(out[ntsl, :], ost[:])

        # ================= emission ======================================
        for jt in range(NT):
            emit_nat_tile(jt, wk_sb, bk_row if has_bias else None, k_nat)
            emit_knorm(jt)
        for jt in range(NT):
            emit_nat_tile(jt, wv_sb, bv_row if has_bias else None, v)
            emit_vscale(jt)

        # pair loop software-pipelined: pair pr's M/W' matmuls are emitted
        # AFTER pair pr+1's norm chain so they (and pr+2's q projection)
        # fill the PE while that chain runs on Pool/DVE/ACT
        for ic in range(NC):
            emit_q_chunk(0, wf0, ic)
        wf = emit_wq_dma(1)
        for ic in range(NC):
            emit_q_chunk(1, wf, ic)
        emit_qnorm(0)
        for pr in range(PAIRS):
            if pr + 2 < PAIRS:
                wf = emit_wq_dma(pr + 2)
                for ic in range(NC):
                    emit_q_chunk(pr + 2, wf, ic)
            if pr + 1 < PAIRS:
                emit_qnorm(pr + 1)
            emit_m(pr)
            emit_wp(pr)
        for ic in range(NC):
            emit_out_chunk(ic)
    return nc


_CACHE = {}


def get_nc(n=2048, has_bias=True):
    key = (n, has_bias)
    if key not in _CACHE:
        nc = bacc.Bacc("TRN2", target_bir_lowering=False, debug=False,
                       num_devices=NCORES)
        build_core_program(nc, n, has_bias=has_bias)
        nc.compile()
        _CACHE[key] = nc
    return _CACHE[key]


def _warr(W, sl):
    return np.ascontiguousarray(
        np.asarray(W, np.float32)[:, sl].reshape(KT, P, FG)
        .transpose(1, 0, 2)).astype(ml_dtypes.bfloat16)


def _warr_ft(W, sl):
    return np.ascontiguousarray(
        np.asarray(W, np.float32)[:, sl].reshape(KT, P, PAIRS, P)
        .transpose(1, 2, 0, 3)).astype(ml_dtypes.bfloat16)


_IND = np.zeros((2, P), ml_dtypes.bfloat16)
_IND[0, :HD] = 1.0
_IND[1, HD:] = 1.0
_BLK = np.zeros((P, 2), ml_dtypes.bfloat16)
_BLK[:HD, 0] = 1.0
_BLK[HD:, 1] = 1.0
_ONES = np.ones((1, 512), ml_dtypes.bfloat16)


def make_in_maps(x, Wq, bq, Wk, bk, Wv, bv, Wo, bo, m, has_bias=True):
    n = x.shape[1]
    sig = 1.0 / (1.0 + np.exp(-np.asarray(m, np.float64)))
    scale = np.float64(n) ** sig  # [16] per-head n^sigmoid(m)
    in_maps = []
    for c in range(NCORES):
        bi, g = divmod(c, 2)
        sl = slice(g * FG, (g + 1) * FG)
        hsc = scale[g * (H // G):(g + 1) * (H // G)]  # 8 local heads
        sv = np.empty((P, PAIRS), np.float32)
        for pr in range(PAIRS):
            sv[:HD, pr] = 1.0 / hsc[2 * pr]
            sv[HD:, pr] = 1.0 / hsc[2 * pr + 1]
        xa = np.asarray(x[bi], np.float32)
        NCc = n // 512
        im = {
            "xt": np.ascontiguousarray(
                xa.reshape(NCc, 512, KT, P).transpose(3, 0, 2, 1))
                .astype(ml_dtypes.bfloat16),
            "wq": _warr_ft(Wq, sl), "wk": _warr(Wk, sl), "wv": _warr(Wv, sl),
            "wo": np.ascontiguousarray(
                np.asarray(Wo, np.float32)[sl].reshape(PAIRS, P, F)
                .transpose(1, 0, 2).astype(ml_dtypes.bfloat16)),
            "sinv": sv,
            "cind": _IND,
            "cblk": _BLK,
        }
        if has_bias:
            im["bqr"] = np.asarray(bq, np.float32)[sl].reshape(1, FG).astype(ml_dtypes.bfloat16)
            im["bkr"] = np.asarray(bk, np.float32)[sl].reshape(1, FG).astype(ml_dtypes.bfloat16)
            im["bvr"] = np.asarray(bv, np.float32)[sl].reshape(1, FG).astype(ml_dtypes.bfloat16)
        in_maps.append(im)
    return in_maps


def kernel(x, Wq, bq, Wk, bk, Wv, bv, Wo, bo, m, _trace=False):
    x = np.asarray(x, np.float32)
    b, n, f = x.shape
    has_bias = bool(np.any(np.asarray(bq)) or np.any(np.asarray(bk))
                    or np.any(np.asarray(bv)))
    nc = get_nc(n, has_bias)
    in_maps = make_in_maps(x, Wq, bq, Wk, bk, Wv, bv, Wo, bo, m,
                           has_bias=has_bias)
    res = bass_utils.run_bass_kernel_spmd(nc, in_maps,
                                          core_ids=list(range(NCORES)),
                                          trace=_trace)
    outs = [r["out"] for r in res.results]
    y = np.empty((b, n, f), np.float32)
    for bi in range(b):
        y[bi] = outs[2 * bi].astype(np.float32) + outs[2 * bi + 1].astype(np.float32)
    y += np.asarray(bo, np.float32).reshape(1, 1, f)
    if _trace:
        kernel._last_results = res
    return y


if __name__ == "__main__":
    # build-only smoke test (no device)
    nc = bacc.Bacc("TRN2", target_bir_lowering=False, debug=False,
                   num_devices=NCORES)
    build_core_program(nc, n=int(sys.argv[1]) if len(sys.argv) > 1 else 2048)
    print("build OK")


# revision 68
# speedup vs baseline: 1.5506x; 1.3245x over previous
"""Multi-head cosine self-attention on 8 Trainium2 NeuronCores (Bass/Tile).

Problem: y = MHA(x) with L2-normalized q/k (cosine attention) and per-head
scaling sim / n**sigmoid(m);  x: [4, 2048, 1024], 16 heads of dim 64.

KEY STRUCTURE: there is no softmax, so attention is LINEAR and associates:
    out_h = (q^ k^T / s_h) v = q^ @ (k^T v~) / s_h,   v~[j,:] = v[j,:]/||k_j||
Per head, M_h = k_h^T v~_h is only [64, 64] — the n x n similarity matrix is
never materialized, collapsing attention FLOPs 16x and eliminating the PSUM
eviction traffic that dominated the quadratic formulation.

Sharding: core c handles batch c//2 and head-group c%2 (8 heads = 512 of the
1024 q/k/v features).  Each core computes its partial output
(attn_out_part @ Wo[rows]); the host sums the two partials per batch and adds
bo.  No collectives.

Per-core pipeline (bf16 operands, fp32 PSUM):
  - k = x Wk + bk in NATURAL layout (rows j on partitions) like v; per-row
    norms ||k_j||_h via GpSimd square + DVE segmented reduce; 1/||k_j|| via
    DVE reciprocal + ACT Sqrt
  - v~ = v * (1/||k||) per head segment via one broadcast tensor_tensor per
    row-tile (stride-0 innermost AP)
  - M_h = k_h^T v~_h: 16 accumulating [K=128,M=64,N=64] matmuls per head,
    2 heads col-packed per PSUM tile; evicted with ACT scale 1/s_h (and the
    per-head n^sigmoid(m) folded in)
  - qT = (x Wq + bq)^T (features on partitions, 2 heads per 128-partition
    tile); normalized in place via the [2,n] column-norm matmul trick,
    reciprocal+sqrt, a K=2 indicator broadcast matmul, and DVE multiply
  - attn_out^T = M_h^T @ q^T: M stationary, q^T streams N=512; two heads
    row+col packed per PSUM tile -> same transposed layout the output
    projection expects
  - out = attn_out^T.T @ Wo_rows, evicted and DMA'd per 128-row tile
Biases are added with K=1 outer-product matmuls inside each projection's
accumulation group, so every PSUM eviction is a plain copy assigned to DVE or
ACT by a greedy running-load balance.
"""

import os
import sys

for _p in ("/opt/trn_rl_repo",):
    if os.path.isdir(_p) and _p not in sys.path:
        sys.path.insert(0, _p)

from contextlib import ExitStack

import ml_dtypes
import numpy as np

import concourse.bacc as bacc
import concourse.mybir as mybir
import concourse.tile as tile
from concourse import bass_utils

P = 128
F = 1024  # model dim
H = 16  # total heads
HD = 64  # head dim
G = 2  # head groups (tensor-parallel factor)
FG = F // G  # 512 features per core
PAIRS = FG // P  # 4 head-pairs per core
NSEG = FG // HD  # 8 head segments per core
KT = F // P  # 8 contraction tiles for the projections
NCORES = 8
F32 = mybir.dt.float32
BF = mybir.dt.bfloat16
AF = mybir.ActivationFunctionType
MUL = mybir.AluOpType.mult


def _mm(nc, out, lhsT, rhs, **kw):
    nc.tensor.matmul(out, lhsT, rhs, **kw)


def build_core_program(nc, n=2048, has_bias=True):
    NC = n // 512  # 512-row chunks
    NT = n // P  # 128-row tiles

    xt = nc.dram_tensor("xt", [P, NC, KT, 512], BF, kind="ExternalInput").ap()
    wq = nc.dram_tensor("wq", [P, PAIRS, KT, P], BF, kind="ExternalInput").ap()
    wk = nc.dram_tensor("wk", [P, KT, FG], BF, kind="ExternalInput").ap()
    wv = nc.dram_tensor("wv", [P, KT, FG], BF, kind="ExternalInput").ap()
    wo = nc.dram_tensor("wo", [P, PAIRS, F], BF, kind="ExternalInput").ap()
    if has_bias:
        bqr = nc.dram_tensor("bqr", [1, FG], BF, kind="ExternalInput").ap()
        bkr = nc.dram_tensor("bkr", [1, FG], BF, kind="ExternalInput").ap()
        bvr = nc.dram_tensor("bvr", [1, FG], BF, kind="ExternalInput").ap()
    # sinv[p, pr] = n**-sigmoid(m_h) for local head h = 2*pr + (p >= 64)
    sinv = nc.dram_tensor("sinv", [P, PAIRS], F32, kind="ExternalInput").ap()
    cind = nc.dram_tensor("cind", [2, P], BF, kind="ExternalInput").ap()
    cblk = nc.dram_tensor("cblk", [P, 2], BF, kind="ExternalInput").ap()
    out = nc.dram_tensor("out", [n, F], BF, kind="ExternalOutput").ap()

    with tile.TileContext(nc) as tc, ExitStack() as ctx:
        const = ctx.enter_context(tc.tile_pool(name="const", bufs=1))
        persist = ctx.enter_context(tc.tile_pool(name="persist", bufs=1))
        work = ctx.enter_context(tc.tile_pool(name="work", bufs=1))
        ps = ctx.enter_context(tc.tile_pool(name="ps", bufs=1, space="PSUM"))

        # --- greedy DVE/ACT load balance for PSUM evictions --------------
        load = {"dve": 0.0, "act": 0.0}

        def copy_ps(dst, src, fd):
            dve_c = (120 + fd) / 0.96
            act_c = (172 + fd) / 1.2
            if load["dve"] + dve_c <= load["act"] + act_c:
                load["dve"] += dve_c
                nc.vector.tensor_copy(dst, src)
            else:
                load["act"] += act_c
                nc.scalar.copy(dst, src)

        # --- persistent tensors ------------------------------------------
        qT = persist.tile([P, PAIRS, n], BF)  # (x Wq + bq)^T, 2 heads/tile
        k_nat = persist.tile([P, NT, FG], BF)  # x Wk + bk, natural layout
        v = persist.tile([P, NT, FG], BF)  # (x Wv + bv) / ||k|| per segment
        wP = persist.tile([P, PAIRS, F], BF)  # W' = M^ @ Wo per pair
        rk = persist.tile([P, NSEG, NT], BF)  # 1/||k_j|| per head segment
        # M^ = (k^T v~)/s as a block-diagonal [128,128] per pair (off-diag
        # zeroed) so attn-out is a single unpacked K=128 matmul
        mh = persist.tile([P, PAIRS, P], BF)
        xall = persist.tile([P, NC, KT, 512], BF)
        wk_sb = persist.tile([P, KT, FG], BF)
        wv_sb = persist.tile([P, KT, FG], BF)
        wo_sb = persist.tile([P, PAIRS, F], BF)

        def emit_wq_dma(pr):
            wf = work.tile([P, KT, P], BF, tag="wqf", bufs=2)
            nc.sync.dma_start(wf[:], wq[:, pr])
            return wf

        # --- constants with no DMA dependency ----------------------------
        ones512 = const.tile([1, 512], BF)
        nc.any.memset(ones512[:], 1.0)
        zcol = const.tile([P, 1], F32)
        nc.any.memset(zcol[:], 0.0)
        nc.any.memset(mh[:], 0.0)  # off-diagonal blocks stay zero

        # --- loads: two queues, strict need-order.  The first k-nat tile
        # needs wk + x chunk 0 + bk_row (1 KB, slotted between the x0
        # halves); every later tensor queues behind in need-order so it
        # does not steal HBM bandwidth from the critical set.  Tiny const
        # DMAs pay ~1us SWDGE first-byte each, so they ride at the back —
        # all are consumed far later than the total DMA span.
        ones_blk = const.tile([P, 2], BF)
        ind = const.tile([2, P], BF)
        sinv_sb = const.tile([P, PAIRS], F32)
        if has_bias:
            bq_row = const.tile([1, FG], BF)
            bk_row = const.tile([1, FG], BF)
            bv_row = const.tile([1, FG], BF)

        quart = KT // 4
        half = KT // 2
        nc.sync.dma_start(wk_sb[:, :quart], wk[:, :quart])
        nc.scalar.dma_start(xall[:, 0, :quart], xt[:, 0, :quart])
        nc.sync.dma_start(wk_sb[:, quart:half], wk[:, quart:half])
        nc.scalar.dma_start(xall[:, 0, quart:half], xt[:, 0, quart:half])
        if has_bias:
            nc.scalar.dma_start(bk_row[:], bkr)
        nc.sync.dma_start(wk_sb[:, half:], wk[:, half:])
        nc.scalar.dma_start(xall[:, 0, half:], xt[:, 0, half:])
        nc.sync.dma_start(xall[:, 1], xt[:, 1])
        nc.scalar.dma_start(xall[:, 2], xt[:, 2])
        nc.sync.dma_start(xall[:, 3], xt[:, 3])
        nc.scalar.dma_start(wv_sb[:], wv)
        if has_bias:
            nc.scalar.dma_start(bv_row[:], bvr)
        wf0 = emit_wq_dma(0)
        nc.scalar.dma_start(wo_sb[:], wo)
        if has_bias:
            nc.sync.dma_start(bq_row[:], bqr)
        nc.sync.dma_start(ones_blk[:], cblk)
        nc.sync.dma_start(ind[:], cind)
        nc.sync.dma_start(sinv_sb[:], sinv)

        def emit_nat_tile(jt, w_sb, brow, dst):
            # one 128-row tile of a natural-layout projection x W + b
            ic, t = divmod(jt, NT // NC)
            jsl = slice(t * P, (t + 1) * P)
            pt = ps.tile([P, FG], F32, tag="mm", bufs=5)
            for k in range(KT):
                _mm(nc, pt, xall[:, ic, k, jsl], w_sb[:, k, :],
                    start=(k == 0), stop=(not has_bias and k == KT - 1))
            if has_bias:
                _mm(nc, pt, ones512[:, :P], brow, start=False, stop=True)
            copy_ps(dst[:, jt, :], pt, 512)

        def emit_knorm(jt):
            # 1/||k_j|| per head segment for one 128-row tile
            ksq = work.tile([P, FG], BF, tag="ksq", bufs=2)
            nc.gpsimd.tensor_tensor(ksq[:], k_nat[:, jt, :], k_nat[:, jt, :], MUL)
            nksq = work.tile([P, NSEG], F32, tag="nksq", bufs=2)
            nc.vector.tensor_reduce(nksq[:],
                                    ksq[:].rearrange("p (s d) -> p s d", s=NSEG),
                                    mybir.AxisListType.X, mybir.AluOpType.add)
            load["dve"] += 658
            rk1 = work.tile([P, NSEG], F32, tag="rk1", bufs=2)
            nc.vector.reciprocal(rk1[:], nksq[:])
            load["dve"] += 140
            nc.scalar.activation(rk[:, :, jt], rk1, AF.Sqrt, bias=zcol[:])
            load["act"] += 200

        def emit_vscale(jt):
            # v~[j,:] = v[j,:] * rk[j, seg] with a stride-0 innermost view
            rkb = rk[:, :, jt:jt + 1].to_broadcast([P, NSEG, HD])
            vv = v[:, jt, :].rearrange("p (s d) -> p s d", s=NSEG)
            if jt % 2 == 0:
                nc.vector.tensor_tensor(vv, vv, rkb, MUL)
                load["dve"] += 690
            else:
                nc.gpsimd.tensor_tensor(vv, vv, rkb, MUL)

        def emit_q_chunk(pr, wf, ic):
            # transposed q projection of pair pr for one 512-row chunk
            isl = slice(ic * 512, (ic + 1) * 512)
            pt = ps.tile([P, 512], F32, tag="mm", bufs=5)
            for k in range(KT):
                _mm(nc, pt, wf[:, k, :], xall[:, ic, k, :],
                    start=(k == 0), stop=(not has_bias and k == KT - 1))
            if has_bias:
                _mm(nc, pt, bq_row[:, pr * P:(pr + 1) * P], ones512,
                    start=False, stop=True)
            copy_ps(qT[:, pr, isl], pt, 512)

        def emit_qnorm(pr):
            # in-place q^ = q/||q|| via the column-norm matmul trick;
            # the square is chunked and alternates Pool/DVE to keep the
            # chain latency off the PE critical path
            sq = work.tile([P, n], BF, tag="sq", bufs=2)
            rowr = work.tile([2, n], BF, tag="rowr", bufs=2)
            for ch in range(NC):
                csl = slice(ch * 512, (ch + 1) * 512)
                if ch % 2 == 0:
                    nc.gpsimd.tensor_tensor(sq[:, csl], qT[:, pr, csl],
                                            qT[:, pr, csl], MUL)
                else:
                    nc.vector.tensor_tensor(sq[:, csl], qT[:, pr, csl],
                                            qT[:, pr, csl], MUL)
                    load["dve"] += 424
                nps = ps.tile([2, 512], F32, tag="av", bufs=2)
                _mm(nc, nps, ones_blk, sq[:, csl], start=True, stop=True)
                row = work.tile([2, 512], F32, tag="row", bufs=2)
                nc.vector.reciprocal(row[:], nps)
                load["dve"] += 658
                nc.scalar.activation(rowr[:, csl], row, AF.Sqrt, bias=zcol[:2])
                load["act"] += 570
            for ch in range(NC):
                csl = slice(ch * 512, (ch + 1) * 512)
                bps = ps.tile([P, 512], F32, tag="av", bufs=2)
                _mm(nc, bps, ind, rowr[:, csl], start=True, stop=True)
                nc.vector.tensor_tensor(qT[:, pr, csl], qT[:, pr, csl],
                                        bps, MUL)
                load["dve"] += 690

        def emit_m(pr):
            # whole-pair M^T = v~_pair^T k_pair in one K=128 matmul per row
            # tile; the useful blocks are the diagonal ones — the
            # off-diagonal junk is discarded at eviction
            mps = ps.tile([P, P], F32, tag="mps", bufs=1)
            psl = slice(pr * P, (pr + 1) * P)
            for jt in range(NT):
                _mm(nc, mps, v[:, jt, psl], k_nat[:, jt, psl],
                    start=(jt == 0), stop=(jt == NT - 1))
            # evictions fold the per-head 1/n**sigmoid(m) scale
            nc.scalar.activation(mh[0:HD, pr, 0:HD], mps[0:HD, 0:HD],
                                 AF.Identity, bias=zcol[:HD],
                                 scale=sinv_sb[0:HD, pr:pr + 1])
            nc.scalar.activation(mh[HD:P, pr, HD:P], mps[HD:P, HD:P],
                                 AF.Identity, bias=zcol[HD:P],
                                 scale=sinv_sb[HD:P, pr:pr + 1])
            load["act"] += 360

        def emit_wp(pr):
            # fold M^ into the output weights: W'_pr = M^_pr @ Wo_pr.
            # out = aoT^T Wo = (M^T qT)^T Wo = qT^T (M^ Wo) — the attn-out
            # stage and its evictions vanish; mh holds M^T block-diagonal
            # so lhsT = mh gives the d-contraction directly.
            for fc in range(F // 512):
                fsl = slice(fc * 512, (fc + 1) * 512)
                wps = ps.tile([P, 512], F32, tag="av", bufs=2)
                _mm(nc, wps, mh[:, pr, :], wo_sb[:, pr, fsl],
                    start=True, stop=True)
                copy_ps(wP[:, pr, fsl], wps, 512)

        def emit_out_chunk(ic):
            # final projection for one 512-row chunk; DMA queues alternate
            for t in range(NT // NC):
                nt = ic * (NT // NC) + t
                ntsl = slice(nt * P, (nt + 1) * P)
                ost = work.tile([P, F], BF, tag="ost", bufs=3)
                for fc in range(F // 512):
                    fsl = slice(fc * 512, (fc + 1) * 512)
                    pt2 = ps.tile([P, 512], F32, tag="mm", bufs=5)
                    for kt in range(PAIRS):
                        _mm(nc, pt2, qT[:, kt, ntsl], wP[:, kt, fsl],
                            start=(kt == 0), stop=(kt == PAIRS - 1))
                    copy_ps(ost[:, fsl], pt2, 512)
                if nt == NT - 1:
                    # final tile: halve the trailing DMA via two queues
                    nc.sync.dma_start(out[ntsl, :512], ost[:, :512])
                    nc.scalar.dma_start(out[ntsl, 512:], ost[:, 512:])
                else:
                    eng = (nc.sync, nc.scalar, nc.gpsimd)[nt % 3]
                    eng.dma_start(out[ntsl, :], ost[:])

        # ================= emission ======================================
        for jt in range(NT):
            emit_nat_tile(jt, wk_sb, bk_row if has_bias else None, k_nat)
            emit_knorm(jt)
        for jt in range(NT):
            emit_nat_tile(jt, wv_sb, bv_row if has_bias else None, v)
            emit_vscale(jt)

        # pair loop software-pipelined: pair pr's M/W' matmuls are emitted
        # AFTER pair pr+1's norm chain so they (and pr+2's q projection)
        # fill the PE while that chain runs on Pool/DVE/ACT
        for ic in range(NC):
            emit_q_chunk(0, wf0, ic)
        wf = emit_wq_dma(1)
        for ic in range(NC):
            emit_q_chunk(1, wf, ic)
        emit_qnorm(0)
        for pr in range(PAIRS):
            if pr + 2 < PAIRS:
                wf = emit_wq_dma(pr + 2)
                for ic in range(NC):
                    emit_q_chunk(pr + 2, wf, ic)
            if pr + 1 < PAIRS:
                emit_qnorm(pr + 1)
            emit_m(pr)
            emit_wp(pr)
        for ic in range(NC):
            emit_out_chunk(ic)
    return nc


_CACHE = {}


def get_nc(n=2048, has_bias=True):
    key = (n, has_bias)
    if key not in _CACHE:
        nc = bacc.Bacc("TRN2", target_bir_lowering=False, debug=False,
                       num_devices=NCORES)
        build_core_program(nc, n, has_bias=has_bias)
        nc.compile()
        _CACHE[key] = nc
    return _CACHE[key]


def _warr(W, sl):
    return np.ascontiguousarray(
        np.asarray(W, np.float32)[:, sl].reshape(KT, P, FG)
        .transpose(1, 0, 2)).astype(ml_dtypes.bfloat16)


def _warr_ft(W, sl):
    return np.ascontiguousarray(
        np.asarray(W, np.float32)[:, sl].reshape(KT, P, PAIRS, P)
        .transpose(1, 2, 0, 3)).astype(ml_dtypes.bfloat16)


_IND = np.zeros((2, P), ml_dtypes.bfloat16)
_IND[0, :HD] = 1.0
_IND[1, HD:] = 1.0
_BLK = np.zeros((P, 2), ml_dtypes.bfloat16)
_BLK[:HD, 0] = 1.0
_BLK[HD:, 1] = 1.0
_ONES = np.ones((1, 512), ml_dtypes.bfloat16)


def make_in_maps(x, Wq, bq, Wk, bk, Wv, bv, Wo, bo, m, has_bias=True):
    n = x.shape[1]
    sig = 1.0 / (1.0 + np.exp(-np.asarray(m, np.float64)))
    scale = np.float64(n) ** sig  # [16] per-head n^sigmoid(m)
    in_maps = []
    for c in range(NCORES):
        bi, g = divmod(c, 2)
        sl = slice(g * FG, (g + 1) * FG)
        hsc = scale[g * (H // G):(g + 1) * (H // G)]  # 8 local heads
        sv = np.empty((P, PAIRS), np.float32)
        for pr in range(PAIRS):
            sv[:HD, pr] = 1.0 / hsc[2 * pr]
            sv[HD:, pr] = 1.0 / hsc[2 * pr + 1]
        xa = np.asarray(x[bi], np.float32)
        NCc = n // 512
        im = {
            "xt": np.ascontiguousarray(
                xa.reshape(NCc, 512, KT, P).transpose(3, 0, 2, 1))
                .astype(ml_dtypes.bfloat16),
            "wq": _warr_ft(Wq, sl), "wk": _warr(Wk, sl), "wv": _warr(Wv, sl),
            "wo": np.ascontiguousarray(
                np.asarray(Wo, np.float32)[sl].reshape(PAIRS, P, F)
                .transpose(1, 0, 2).astype(ml_dtypes.bfloat16)),
            "sinv": sv,
            "cind": _IND,
            "cblk": _BLK,
        }
        if has_bias:
            im["bqr"] = np.asarray(bq, np.float32)[sl].reshape(1, FG).astype(ml_dtypes.bfloat16)
            im["bkr"] = np.asarray(bk, np.float32)[sl].reshape(1, FG).astype(ml_dtypes.bfloat16)
            im["bvr"] = np.asarray(bv, np.float32)[sl].reshape(1, FG).astype(ml_dtypes.bfloat16)
        in_maps.append(im)
    return in_maps


def kernel(x, Wq, bq, Wk, bk, Wv, bv, Wo, bo, m, _trace=False):
    x = np.asarray(x, np.float32)
    b, n, f = x.shape
    has_bias = bool(np.any(np.asarray(bq)) or np.any(np.asarray(bk))
                    or np.any(np.asarray(bv)))
    nc = get_nc(n, has_bias)
    in_maps = make_in_maps(x, Wq, bq, Wk, bk, Wv, bv, Wo, bo, m,
                           has_bias=has_bias)
    res = bass_utils.run_bass_kernel_spmd(nc, in_maps,
                                          core_ids=list(range(NCORES)),
                                          trace=_trace)
    outs = [r["out"] for r in res.results]
    y = np.empty((b, n, f), np.float32)
    for bi in range(b):
        y[bi] = outs[2 * bi].astype(np.float32) + outs[2 * bi + 1].astype(np.float32)
    y += np.asarray(bo, np.float32).reshape(1, 1, f)
    if _trace:
        kernel._last_results = res
    return y


if __name__ == "__main__":
    # build-only smoke test (no device)
    nc = bacc.Bacc("TRN2", target_bir_lowering=False, debug=False,
                   num_devices=NCORES)
    build_core_program(nc, n=int(sys.argv[1]) if len(sys.argv) > 1 else 2048)
    print("build OK")
